# revision 32
# baseline (speedup 1.0000x reference)
"""Trainium2 Bass kernel for per-node LocalConv1D (kernel_size=1).

out[b, o, n] = sum_h W[n, o, h] * x[b, h, n] + b[n, o]

Full shapes: x [16, 32, 50000] f32, W [50000, 32, 32] f32, b [50000, 32] f32,
out [16, 32, 50000] f32.

Sharding: node dim n split evenly across 8 NeuronCores (6250 nodes/core,
zero-padded to 6272 = 98*64 inside each shard). Fully independent per-node
32x32 matmuls -> no collectives.

Device strategy (v2, x-stationary):
  The PE weight path (LDWEIGHTS) costs ~1 cycle per stationary COLUMN, so we
  make x the stationary operand: one LDWEIGHTS of [32h x 32] covers the
  16-batch columns of TWO nodes (a "pair"), and each pair is ONE InstMatmult
  whose moving operand is the two nodes' weights [32h x 64o] streamed from
  SBUF on the other read port. All tensors are bf16 (tolerance 2e-2; bf16
  gives ~4e-3). PSUM result per pair is [32 x 64] where for node i of the
  pair only rows i*16..i*16+16 are valid (the other 16 rows are the cross
  product of the wrong node's x and are discarded on the host).

  PE 32x32 tiling, DIAGONAL tiles only: quadrant r computes on tile
  (32r, 32r), so its PSUM lands in partition band r. (Driving all 16
  cross tiles into a single PSUM bank crashes the device; <=4 concurrent
  tile streams per bank is safe.) A round = 64 nodes (16 per quadrant) =
  32 matmuls = one PSUM bank [128 x 512 f32]; 98 rounds, 14 W slabs of 7
  rounds. All evictions run on DVE as PSUM->SBUF bf16 copies whose APs
  regroup f to (i, k, o) so each node-half is f-contiguous; the out DMA
  (ACT engine) then ships only the valid 16-partition strips per (r, i)
  in 512B runs. W DMAs ride SP, x chunk DMAs ride gpsimd, prefetched two
  slabs ahead. Two BIR post-passes: _drop_redundant_pe_self_waits removes
  Tile's implied PE-progress waits (each one drains the PE pipeline), and
  _legalize_waits splits multi-wait sync structs for walrus.

  Bias is added on the host during reassembly (out partitions are batch
  lanes on-device, so a device-side bias add would need a partition
  broadcast the vector engines don't have).
"""

from contextlib import ExitStack

import numpy as np

import concourse.bass as bass
import concourse.mybir as mybir
import concourse.tile as tile
from concourse.ap import AP


def _ap(handle_ap, offset, dims):
    """Raw AP on the same tensor: dims = [(step, count), ...] in elements."""
    return AP(handle_ap.tensor, offset, [[int(s), int(c)] for s, c in dims])

F32 = mybir.dt.float32
BF16 = mybir.dt.bfloat16

B = 16  # batch
H = 32  # in channels
O = 32  # out channels
NCORES = 8
NFULL = 50000
NPC = NFULL // NCORES  # 6250 nodes per core
NPAD = 6272  # 98 * 64, per-core padded node count
Q = NPAD // 4  # 1568 nodes per quadrant (row group)
NPR = 16  # nodes per quadrant per round
ROUNDS = Q // NPR  # 98
RPS = 7  # rounds per W/out slab
NSLAB = ROUNDS // RPS  # 14
RPX = 14  # rounds per x chunk
NXT = ROUNDS // RPX  # 7 x chunks
XF = RPX * NPR * B  # 3584 x f-columns per chunk
WF = RPS * NPR * O  # 3584 W f-columns per slab
OF = RPS * 512  # 3584 out f-columns per slab
OUT_F = ROUNDS * 256  # 25088 (valid-only)


def build_bass():
    nslab_run = NSLAB
    nxt_run = NXT
    nc = bass.Bass()
    x_d = nc.declare_dram_parameter("x", [128, Q * B], BF16, isOutput=False)
    w_d = nc.declare_dram_parameter("W", [128, Q * O], BF16, isOutput=False)
    out_d = nc.declare_dram_parameter("out", [128, OUT_F], BF16, isOutput=True)

    with ExitStack() as ctx:
        tc = ctx.enter_context(tile.TileContext(nc))
        xtp = ctx.enter_context(tc.tile_pool(name="xtp", bufs=NXT))
        wtp = ctx.enter_context(tc.tile_pool(name="wtp", bufs=4))
        outp = ctx.enter_context(tc.tile_pool(name="outp", bufs=3))
        psp = ctx.enter_context(tc.tile_pool(name="psp", bufs=6, space="PSUM"))

        # resident x chunks, prefetched one slab ahead of first use
        xts = []

        def load_x(t):
            xt = xtp.tile([128, XF], BF16)
            nc.gpsimd.dma_start(out=xt[:], in_=x_d[:, t * XF : (t + 1) * XF])
            xts.append(xt)

        load_x(0)

        for sl in range(nslab_run):
            want = min((sl + 4) // 2, nxt_run - 1)  # two slabs ahead
            while len(xts) <= want:
                load_x(len(xts))
            wt = wtp.tile([128, WF], BF16)
            nc.sync.dma_start(out=wt[:], in_=w_d[:, sl * WF : (sl + 1) * WF])
            if sl % 2 == 0:
                ot = outp.tile([128, 2 * OF], BF16)

            for lr in range(RPS):
                g = sl * RPS + lr
                xt = xts[min(g // RPX, nxt_run - 1)]
                xbase = (g % RPX) * NPR * B  # start f of this round's nodes
                ps = psp.tile([128, 512], F32)
                ps_v = ps[:]
                x_v = xt[:]
                w_v = wt[:]
                for k in range(8):
                    for r in range(4):
                        nc.tensor.matmul(
                            ps_v[32 * r : 32 * r + 32, k * 64 : k * 64 + 64],
                            x_v[32 * r : 32 * r + 32,
                                xbase + k * 2 * B : xbase + k * 2 * B + 2 * B],
                            w_v[32 * r : 32 * r + 32,
                                lr * 512 + k * 2 * O : lr * 512 + k * 2 * O + 2 * O],
                            start=True,
                            stop=True,
                            tile_position=(32 * r, 32 * r),
                        )
                dst = (
                    ot[:]
                    .rearrange("p (s lr i ko) -> p s lr i ko", s=2, lr=RPS, i=2, ko=256)
                    [:, sl % 2, lr]
                )
                srcv = ps[:].rearrange("p (k i o) -> p i k o", k=8, i=2, o=O)
                dstv = dst.rearrange("p i (k o) -> p i k o", k=8, o=O)
                nc.vector.tensor_copy(out=dstv, in_=srcv)

            if sl % 2 == 1:
                g0 = (sl - 1) * RPS  # first round in this 2-slab group
                for r in range(4):
                    for i in range(2):
                        srcp = (
                            ot[32 * r + 16 * i : 32 * r + 16 * i + 16]
                            .rearrange(
                                "p (s lr i ko) -> p s lr i ko",
                                s=2, lr=RPS, i=2, ko=256,
                            )[:, :, :, i, :]
                        )
                        dstp = _ap(
                            out_d[:],
                            (i * 64 + r * 16) * OUT_F + g0 * 256,
                            [(OUT_F, 16), (256, 2 * RPS), (1, 256)],
                        )
                        nc.scalar.dma_start(out=dstp, in_=srcp)

    return nc


def _drop_redundant_pe_self_waits(nc):
    """Tile guards PSUM-bank WAW with a matmul-count wait (PE_*) on top of
    the eviction-count wait (DVE_*). The eviction it waits for itself waited
    for those matmuls (32 per round), so the PE self-wait is implied — and
    materializing it as a PE EventSemaphore drains the PE pipeline every
    round. Drop PE-engine waits on the PE progress sem whenever the same
    instruction also waits on the DVE progress sem with 32*dve >= pe."""
    ndrop = 0
    for f in nc.m.functions:
        for bb in f.blocks:
            for inst in bb.instructions:
                if str(inst.engine) != "EngineType.PE":
                    continue
                si = getattr(inst, "sync_info", None)
                if si is None or len(si.on_wait) < 2:
                    continue
                dve = [w for w in si.on_wait if w.ant_name.startswith("DVE_")]
                keep = []
                for w in si.on_wait:
                    if (
                        w.ant_name.startswith("PE_")
                        and any(32 * d.wait_value >= w.wait_value for d in dve)
                    ):
                        ndrop += 1
                        continue
                    keep.append(w)
                if len(keep) != len(si.on_wait):
                    inst.sync_info = mybir.SyncInfo(
                        on_wait=keep, on_update=list(si.on_update)
                    )
    return nc


def _legalize_waits(nc):
    """Walrus's per-instruction sync structs carry at most one wait
    (DMA_DIRECT2D, S3_LW, ...); Tile sometimes leaves several on one
    instruction. Move the surplus onto EventSemaphore instructions inserted
    just before it on the same engine — the issuing sequencer executes its
    stream in order, so the waits still gate the instruction."""
    nsplit = 0
    for f in nc.m.functions:
        for bb in f.blocks:
            new = []
            changed = False
            for inst in bb.instructions:
                si = getattr(inst, "sync_info", None)
                if (
                    si is not None
                    and si.on_wait
                    and len(si.on_wait) > 1
                    and type(inst).__name__ != "InstEventSemaphore"
                ):
                    waits = list(si.on_wait)
                    for w in waits[:-1]:
                        nsplit += 1
                        new.append(
                            mybir.InstEventSemaphore(
                                name=f"wait-split-{nsplit}",
                                engine=inst.engine,
                                ins=[],
                                outs=[],
                                sync_info=mybir.SyncInfo(
                                    on_wait=[w], on_update=[]
                                ),
                            )
                        )
                    inst.sync_info = mybir.SyncInfo(
                        on_wait=[waits[-1]], on_update=list(si.on_update)
                    )
                    changed = True
                new.append(inst)
            if changed:
                bb.instructions = new
    return nc


_NC_CACHE = {}


def _get_nc():
    if "nc" not in _NC_CACHE:
        _NC_CACHE["nc"] = _legalize_waits(_drop_redundant_pe_self_waits(build_bass()))
    return _NC_CACHE["nc"]


def prep_core_inputs(x_s, W_s):
    """Per-core shard [*, NPC nodes] -> device-layout bf16 arrays (padded)."""
    import ml_dtypes

    bf16 = ml_dtypes.bfloat16
    xs = np.zeros((B, H, NPAD), bf16)
    xs[:, :, :NPC] = x_s.astype(bf16)
    Ws = np.zeros((NPAD, O, H), bf16)
    Ws[:NPC] = W_s.astype(bf16)

    # x: [p=(r,h), f=(m,b)] ; m is the node index within the quadrant
    xp = (
        xs.reshape(B, H, 4, Q)
        .transpose(2, 1, 3, 0)
        .reshape(128, Q * B)
        .copy()
    )

    # W: [p=(r,h), f=(m,o)] (per-node W transposed to [h, o])
    wp = (
        Ws.reshape(4, Q, O, H)
        .transpose(0, 3, 1, 2)
        .reshape(128, Q * O)
        .copy()
    )

    return {"x": xp, "W": wp}


def unprep_core_output(op):
    """Device out slab [128, OUT_F] bf16 -> [B, O, NPC] f32 (garbage rows
    of each pair stripped)."""
    arr = np.asarray(op).astype(np.float32)
    # p = (i:2, r:4, b:16), f = (g:98, k:8, o:32); n = r*Q + g*16 + k*2 + i
    arr = arr.reshape(2, 4, B, ROUNDS, 8, O)
    out = arr.transpose(2, 5, 1, 3, 4, 0).reshape(B, O, NPAD)
    return out[:, :, :NPC]


def make_in_maps(x, W, b=None):
    x = np.ascontiguousarray(x, dtype=np.float32)
    W = np.ascontiguousarray(W, dtype=np.float32)
    in_maps = []
    for core in range(NCORES):
        sl = slice(core * NPC, (core + 1) * NPC)
        in_maps.append(prep_core_inputs(x[:, :, sl], W[sl]))
    return in_maps


def run_spmd(in_maps, **kwargs):
    from concourse.bass_utils import run_bass_kernel_spmd

    nc = _get_nc()
    return run_bass_kernel_spmd(
        nc, in_maps, core_ids=list(range(NCORES)), **kwargs
    )


def assemble_output(res, b):
    out = np.concatenate(
        [unprep_core_output(res.results[c]["out"]) for c in range(NCORES)],
        axis=2,
    )
    # bias epilogue on host: out[b, o, n] += bias[n, o]
    out += np.ascontiguousarray(b, dtype=np.float32).T[None, :, :]
    return out


def kernel(x, W, b):
    res = run_spmd(make_in_maps(x, W))
    return assemble_output(res, b)
